# revision 9
# baseline (speedup 1.0000x reference)
"""Bayesian MuRP kernel for 8 TRN2 NeuronCores (Bass/Tile, SPMD data-parallel).

Strategy: data-parallel over batch (B=2048 -> 256 rows/core). The huge
lm_hidden_state is only ever read at 2 rows per batch element, so the gather
happens on host and only the gathered rows (~16 MB total) ship to HBM.
BatchNorm needs global batch stats -> tiny [100,2] AllReduce across the 8
cores inside the kernel. Everything else is per-core.
"""

import numpy as np

B, S, H = 2048, 128, 1024
LAT, HID, MD, NR = 50, 300, 100, 4
BN_EPS = 1e-5
DROP_P = 0.5
NCORES = 8
BC = B // NCORES  # 256 rows per core
NH = 2            # batch halves of 128 within a core

TRACE = False          # set True by test harness to capture NTFF profile
LAST_RESULTS = None    # BassKernelResults of the last run (for profiling)

_built = None


def _build():
    """Build + compile the Bass module once per process."""
    global _built
    if _built is not None:
        return _built

    import concourse.bass as bass
    import concourse.mybir as mybir
    import concourse.bacc as bacc
    import concourse.tile as tile

    f32 = mybir.dt.float32
    AF = mybir.ActivationFunctionType
    ALU = mybir.AluOpType
    X = mybir.AxisListType.X

    nc = bacc.Bacc("TRN2", target_bir_lowering=False, debug=False,
                   num_devices=NCORES)

    # ---- DRAM I/O ----
    ee_d = nc.dram_tensor("ee", [128, 8, 2, 256], f32, kind="ExternalInput")
    wev_d = nc.dram_tensor("wev", [128, 8, MD], f32, kind="ExternalInput")
    wfc1_d = nc.dram_tensor("wfc1", [100, 2, 3, 100], f32, kind="ExternalInput")
    wms_d = nc.dram_tensor("wms", [100, 3, 128], f32, kind="ExternalInput")
    wmure_d = nc.dram_tensor("wmure", [LAT, 800], f32, kind="ExternalInput")
    bmure_d = nc.dram_tensor("bmure", [1, 800], f32, kind="ExternalInput")
    maskt_d = nc.dram_tensor("maskt", [100, 3, 256], f32, kind="ExternalInput")
    epst_d = nc.dram_tensor("epst", [LAT, 256], f32, kind="ExternalInput")
    bev_d = nc.dram_tensor("bev", [MD, 1], f32, kind="ExternalInput")
    bfc1_d = nc.dram_tensor("bfc1", [100, 3], f32, kind="ExternalInput")
    bms_d = nc.dram_tensor("bms", [128, 1], f32, kind="ExternalInput")
    gb_d = nc.dram_tensor("gb", [128, 2], f32, kind="ExternalInput")
    ident_d = nc.dram_tensor("ident", [128, 128], f32, kind="ExternalInput")

    logits_d = nc.dram_tensor("logits_o", [BC, NR], f32, kind="ExternalOutput")
    mu_d = nc.dram_tensor("mu_o", [BC, LAT], f32, kind="ExternalOutput")
    lv_d = nc.dram_tensor("lv_o", [BC, LAT], f32, kind="ExternalOutput")

    with tile.TileContext(nc) as tc:
        with (
            tc.tile_pool(name="const", bufs=1) as cp,
            tc.tile_pool(name="act", bufs=1) as ap_,
            tc.tile_pool(name="scr", bufs=2) as scr,
            tc.tile_pool(name="psA", bufs=2, space="PSUM") as psA,
            tc.tile_pool(name="psB", bufs=2, space="PSUM") as psB,
            tc.tile_pool(name="psG", bufs=2, space="PSUM") as psG,
            tc.tile_pool(name="psT", bufs=2, space="PSUM") as psT,
            tc.tile_pool(name="dram", bufs=1, space="DRAM") as dp,
        ):
            # ---------- load constants / weights ----------
            wev = cp.tile([128, 8, MD], f32, tag="wev")
            nc.sync.dma_start(wev[:], wev_d[:])
            wfc1 = cp.tile([100, 2, 3, 100], f32, tag="wfc1")
            nc.sync.dma_start(wfc1[:], wfc1_d[:])
            wms = cp.tile([100, 3, 128], f32, tag="wms")
            nc.sync.dma_start(wms[:], wms_d[:])
            wmure = cp.tile([LAT, 800], f32, tag="wmure")
            nc.sync.dma_start(wmure[:], wmure_d[:])
            bmure = cp.tile([1, 800], f32, tag="bmure")
            nc.sync.dma_start(bmure[:], bmure_d[:])
            ones_t = cp.tile([1, 128], f32, tag="ones_t")
            nc.vector.memset(ones_t[:], 1.0)
            maskt = cp.tile([100, 3, 256], f32, tag="maskt")
            nc.sync.dma_start(maskt[:], maskt_d[:])
            epst = cp.tile([LAT, 256], f32, tag="epst")
            nc.sync.dma_start(epst[:], epst_d[:])
            bev = cp.tile([MD, 1], f32, tag="bev")
            nc.sync.dma_start(bev[:], bev_d[:])
            bfc1 = cp.tile([100, 3], f32, tag="bfc1")
            nc.sync.dma_start(bfc1[:], bfc1_d[:])
            bms = cp.tile([128, 1], f32, tag="bms")
            nc.sync.dma_start(bms[:], bms_d[:])
            gb = cp.tile([128, 2], f32, tag="gb")
            nc.sync.dma_start(gb[:], gb_d[:])
            ident = cp.tile([128, 128], f32, tag="ident")
            nc.sync.dma_start(ident[:], ident_d[:])

            # ---------- stage A: e1p/e2p = gathered rows @ w_event ----------
            ee = [cp.tile([128, 2, 256], f32, tag=f"ee{k}", name=f"ee{k}")
                  for k in range(8)]
            for k in range(8):
                nc.sync.dma_start(ee[k][:], ee_d[:, k, :, :])

            p_e1 = psA.tile([MD, 256], f32, tag="pA")
            p_e2 = psA.tile([MD, 256], f32, tag="pA")
            for k in range(8):
                nc.tensor.matmul(p_e1[:], wev[:, k, :], ee[k][:, 0, :],
                                 start=(k == 0), stop=(k == 7))
            for k in range(8):
                nc.tensor.matmul(p_e2[:], wev[:, k, :], ee[k][:, 1, :],
                                 start=(k == 0), stop=(k == 7))
            e1pt = ap_.tile([MD, 256], f32, tag="e1pt")
            e2pt = ap_.tile([MD, 256], f32, tag="e2pt")
            nc.scalar.activation(e1pt[:], p_e1[:], AF.Identity, bias=bev[:])
            nc.scalar.activation(e2pt[:], p_e2[:], AF.Identity, bias=bev[:])

            # ---------- stage B: reps = relu(cat @ w_fc1 + b) * mask ----------
            repst = ap_.tile([100, 3, 256], f32, tag="repst")
            for j in range(3):
                pf = psB.tile([100, 256], f32, tag="pB")
                nc.tensor.matmul(pf[:], wfc1[:, 0, j, :], e1pt[:],
                                 start=True, stop=False)
                nc.tensor.matmul(pf[:], wfc1[:, 1, j, :], e2pt[:],
                                 start=False, stop=True)
                nc.scalar.activation(repst[:, j, :], pf[:], AF.Relu,
                                     bias=bfc1[:, j:j + 1])
                nc.vector.tensor_mul(repst[:, j, :], repst[:, j, :],
                                     maskt[:, j, :])

            # ---------- stage C: pml = [premu; prelogvar] ----------
            p_ms = psB.tile([128, 256], f32, tag="pB")
            for j in range(3):
                nc.tensor.matmul(p_ms[:], wms[:, j, :], repst[:, j, :],
                                 start=(j == 0), stop=(j == 2))
            pml = ap_.tile([128, 256], f32, tag="pml")
            nc.scalar.activation(pml[:], p_ms[:], AF.Identity, bias=bms[:])

            # ---------- stage D: batch stats partial sums + AllReduce ----------
            sums = ap_.tile([128, 2], f32, tag="sums")
            nc.vector.tensor_reduce(sums[:, 0:1], pml[:], X, ALU.add)
            sq_s = scr.tile([128, 256], f32, tag="sqs")
            nc.scalar.activation(sq_s[:], pml[:], AF.Square,
                                 accum_out=sums[:, 1:2])
            cc_in = dp.tile([128, 2], f32, tag="ccin")
            cc_out = dp.tile([128, 2], f32, tag="ccout")
            nc.sync.dma_start(cc_in[:], sums[:])
            nc.gpsimd.collective_compute(
                "AllReduce", ALU.add,
                replica_groups=[list(range(NCORES))],
                ins=[cc_in.opt()],
                outs=[cc_out.opt()],
            )
            gsums = ap_.tile([128, 2], f32, tag="gsums")
            nc.sync.dma_start(gsums[:], cc_out[:])

            # ---------- stage E: bn affine ----------
            st = ap_.tile([128, 6], f32, tag="st")  # m, ex2, var, sd, a, c
            nc.vector.tensor_scalar_mul(st[:, 0:1], gsums[:, 0:1], 1.0 / B)
            nc.vector.tensor_scalar_mul(st[:, 1:2], gsums[:, 1:2], 1.0 / B)
            nc.vector.tensor_mul(st[:, 2:3], st[:, 0:1], st[:, 0:1])
            nc.vector.tensor_sub(st[:, 2:3], st[:, 1:2], st[:, 2:3])
            nc.vector.tensor_scalar_add(st[:, 2:3], st[:, 2:3], BN_EPS)
            nc.scalar.activation(st[:, 3:4], st[:, 2:3], AF.Sqrt)
            nc.vector.reciprocal(st[:, 3:4], st[:, 3:4])
            nc.vector.tensor_mul(st[:, 4:5], gb[:, 0:1], st[:, 3:4])
            nc.vector.tensor_mul(st[:, 5:6], st[:, 0:1], st[:, 4:5])
            nc.vector.tensor_sub(st[:, 5:6], gb[:, 1:2], st[:, 5:6])
            bn = ap_.tile([128, 256], f32, tag="bn")
            nc.vector.tensor_scalar(bn[:], pml[:], st[:, 4:5], st[:, 5:6],
                                    ALU.mult, ALU.add)

            # mu/logvar outputs via PE transpose
            for h in range(NH):
                ptr = psT.tile([128, 128], f32, tag="pT")
                nc.tensor.transpose(ptr[:], bn[:, h * 128:(h + 1) * 128],
                                    ident[:])
                mlv = scr.tile([128, 128], f32, tag="mlv", name=f"mlv{h}")
                nc.vector.tensor_copy(mlv[:], ptr[:])
                nc.sync.dma_start(mu_d[h * 128:(h + 1) * 128, :],
                                  mlv[:, 0:LAT])
                nc.sync.dma_start(lv_d[h * 128:(h + 1) * 128, :],
                                  mlv[:, 64:64 + LAT])

            # ---------- stage F: zT ----------
            zt = ap_.tile([LAT, 256], f32, tag="zt")
            sig = scr.tile([LAT, 256], f32, tag="sig")
            nc.scalar.activation(sig[:], bn[64:64 + LAT, :], AF.Exp, scale=0.5)
            nc.vector.tensor_mul(zt[:], sig[:], epst[:])
            nc.vector.tensor_add(zt[:], zt[:], bn[0:LAT, :])

            # ---------- stage G: tp (batch-major) + transposes ----------
            tpWu, tprvh, e1p4, e2p4 = [], [], [], []
            for h in range(NH):
                hs = slice(h * 128, (h + 1) * 128)
                tw = ap_.tile([128, 400], f32, tag=f"tpWu{h}")
                tr = ap_.tile([128, 400], f32, tag=f"tprvh{h}")
                for seg, dst in ((0, tw), (1, tr)):
                    pg = psG.tile([128, 400], f32, tag="pG")
                    cs = slice(seg * 400, (seg + 1) * 400)
                    nc.tensor.matmul(pg[:], zt[:, hs], wmure[:, cs],
                                     start=True, stop=False)
                    nc.tensor.matmul(pg[:], ones_t[:], bmure[:, cs],
                                     start=False, stop=True)
                    nc.scalar.activation(dst[:], pg[:], AF.Tanh)
                tpWu.append(tw)
                tprvh.append(tr)

                e14 = ap_.tile([128, 4, 100], f32, tag=f"e1p4_{h}")
                e24 = ap_.tile([128, 4, 100], f32, tag=f"e2p4_{h}")
                for si, (src, dst4) in enumerate(((e1pt, e14), (e2pt, e24))):
                    ptr = psT.tile([128, 100], f32, tag="pT")
                    nc.tensor.transpose(ptr[:], src[:, hs],
                                        ident[0:100, 0:100])
                    bc = ptr[:].unsqueeze(1).broadcast_to([128, 4, 100])
                    if si == 0:
                        nc.vector.tensor_copy(dst4[:], bc)
                    else:
                        nc.scalar.activation(dst4[:], bc, AF.Copy)
                e1p4.append(e14)
                e2p4.append(e24)

            # ---------- stage H: hyperbolic block, per batch half ----------
            def bc4(t4):  # [128,4] -> [128,4,100] broadcast view
                return t4.unsqueeze(2).broadcast_to([128, 4, 100])

            for h in range(NH):
                wu = tpWu[h][:].rearrange("p (r m) -> p r m", r=4)
                rv = tprvh[h][:].rearrange("p (r m) -> p r m", r=4)
                e14, e24 = e1p4[h], e2p4[h]

                sm = ap_.tile([128, 20], f32, tag=f"sm{h}")  # small scalars
                n2v = sm[:, 0:1]
                thv = sm[:, 1:2]
                facv = sm[:, 2:3]
                x2v = sm[:, 3:4]
                cBv = sm[:, 4:5]

                # v = expmap0(e2p)
                sq1 = scr.tile([128, 100], f32, tag=f"sq1_{h}")
                nc.scalar.activation(sq1[:], e24[:, 0, :], AF.Square,
                                     accum_out=n2v)
                nc.scalar.activation(n2v, n2v, AF.Sqrt)          # nv
                nc.vector.tensor_scalar_max(n2v, n2v, 1e-15)     # nvc
                nc.scalar.activation(thv, n2v, AF.Tanh)
                nc.vector.reciprocal(facv, n2v)
                nc.vector.tensor_mul(facv, thv, facv)            # tanh(n)/n
                nc.scalar.activation(x2v, thv, AF.Square)
                nc.vector.tensor_scalar(cBv, x2v, -1.0, 1.0, ALU.mult,
                                        ALU.add)                 # 1-x2v

                s4 = ap_.tile([128, 4, 16], f32, tag=f"s4_{h}")
                n2u = s4[:, :, 0]
                thu = s4[:, :, 1]
                facu = s4[:, :, 2]
                x2u = s4[:, :, 3]
                y2 = s4[:, :, 4]
                sxy = s4[:, :, 5]
                t2s = s4[:, :, 6]
                cA = s4[:, :, 7]
                den = s4[:, :, 8]
                y2p = s4[:, :, 9]
                sp = s4[:, :, 10]
                t2sp = s4[:, :, 11]
                cAp = s4[:, :, 12]
                denp = s4[:, :, 13]
                nn2 = s4[:, :, 14]
                cBp = s4[:, :, 15]

                # u_in = e1p * Wu ; norms per segment
                u_in = ap_.tile([128, 4, 100], f32, tag=f"uin{h}")
                nc.vector.tensor_mul(u_in[:], e14[:], wu)
                sq4 = scr.tile([128, 4, 100], f32, tag=f"sq4_{h}")
                nc.scalar.activation(sq4[:], u_in[:], AF.Square)
                nc.vector.tensor_reduce(n2u, sq4[:], X, ALU.add)
                nc.scalar.activation(n2u, n2u, AF.Sqrt)          # nu
                nc.vector.tensor_scalar_max(n2u, n2u, 1e-15)
                nc.scalar.activation(thu, n2u, AF.Tanh)
                nc.vector.reciprocal(facu, n2u)
                nc.vector.tensor_mul(facu, thu, facu)
                nc.scalar.activation(x2u, thu, AF.Square)
                nc.vector.tensor_scalar(cBp, x2u, -1.0, 1.0, ALU.mult,
                                        ALU.add)                 # 1-x2u

                # v_m = mobius_add(v, rvh); v = facv * e2p (not materialized)
                nc.scalar.activation(sq4[:], rv, AF.Square)
                nc.vector.tensor_reduce(y2, sq4[:], X, ALU.add)
                pr = scr.tile([128, 4, 100], f32, tag=f"pr{h}")
                nc.gpsimd.tensor_mul(pr[:], e24[:], rv)
                nc.vector.tensor_reduce(sxy, pr[:], X, ALU.add)
                nc.vector.tensor_scalar_mul(sxy, sxy, facv)      # xy
                nc.vector.tensor_scalar(t2s, sxy, 2.0, 1.0, ALU.mult,
                                        ALU.add)                 # 1+2xy
                nc.vector.tensor_add(cA, t2s, y2)                # 1+2xy+y2
                nc.vector.tensor_scalar_mul(den, y2, x2v)
                nc.vector.tensor_add(den, den, t2s)              # 1+2xy+x2*y2
                nc.vector.tensor_scalar_max(den, den, 1e-15)
                nc.vector.reciprocal(den, den)                   # 1/den

                nc.vector.tensor_scalar_mul(cA, cA, facv)        # coefA*facv
                num = ap_.tile([128, 4, 100], f32, tag=f"num{h}")
                nc.gpsimd.tensor_mul(num[:], e24[:], bc4(cA))    # coefA*v
                vm = ap_.tile([128, 4, 100], f32, tag=f"vm{h}")
                nc.vector.tensor_scalar_mul(vm[:], rv, cBv)      # coefB*y
                nc.vector.tensor_add(num[:], num[:], vm[:])
                nc.vector.tensor_mul(vm[:], num[:], bc4(den))    # v_m

                # pdist(u_m, v_m); u_m = facu (x) u_in (not materialized)
                nc.scalar.activation(sq4[:], vm[:], AF.Square)
                nc.vector.tensor_reduce(y2p, sq4[:], X, ALU.add)
                nc.gpsimd.tensor_mul(pr[:], u_in[:], vm[:])
                nc.vector.tensor_reduce(sp, pr[:], X, ALU.add)
                nc.vector.tensor_mul(sp, sp, facu)               # u_m . v_m
                nc.vector.tensor_scalar(t2sp, sp, -2.0, 1.0, ALU.mult,
                                        ALU.add)                 # 1+2xyp
                nc.vector.tensor_add(cAp, t2sp, y2p)
                nc.vector.tensor_mul(denp, x2u, y2p)
                nc.vector.tensor_add(denp, denp, t2sp)
                nc.vector.tensor_scalar_max(denp, denp, 1e-15)
                nc.vector.reciprocal(denp, denp)

                nc.vector.tensor_mul(cAp, cAp, facu)             # coefAp*facu
                # num_p = coefBp*v_m - coefAp*u_m
                nc.gpsimd.tensor_mul(num[:], u_in[:], bc4(cAp))
                nc.vector.tensor_mul(vm[:], vm[:], bc4(cBp))
                nc.vector.tensor_sub(num[:], vm[:], num[:])
                nc.scalar.activation(sq4[:], num[:], AF.Square)
                nc.vector.tensor_reduce(nn2, sq4[:], X, ALU.add)

                lg = ap_.tile([128, 4, 4], f32, tag=f"lg{h}")
                n_ = lg[:, :, 0]
                p_ = lg[:, :, 1]
                q_ = lg[:, :, 2]
                o_ = lg[:, :, 3]
                nc.scalar.activation(n_, nn2, AF.Sqrt)
                nc.vector.tensor_mul(n_, n_, denp)               # n = |num|/den
                nc.vector.tensor_scalar_min(n_, n_, 1.0 - 1e-5)
                nc.vector.tensor_scalar(p_, n_, 1.0, 1.0, ALU.mult, ALU.add)
                nc.vector.tensor_scalar(q_, n_, -1.0, 1.0, ALU.mult, ALU.add)
                nc.vector.reciprocal(q_, q_)
                nc.vector.tensor_mul(p_, p_, q_)
                nc.scalar.activation(o_, p_, AF.Ln)
                nc.sync.dma_start(logits_d[h * 128:(h + 1) * 128, :], o_)

    nc.compile()
    _built = nc
    return nc


def _prep_inputs(inputs):
    lm = np.asarray(inputs["lm_hidden_state"], dtype=np.float32)
    ep1 = np.asarray(inputs["epos_1"]).astype(np.int64)
    ep2 = np.asarray(inputs["epos_2"]).astype(np.int64)
    eps = np.asarray(inputs["eps"], dtype=np.float32)
    drop_u = np.asarray(inputs["drop_u"], dtype=np.float32)

    bidx = np.arange(B)
    e1 = lm[bidx, ep1]          # [B, H]
    e2 = lm[bidx, ep2]
    mask = (drop_u > DROP_P).astype(np.float32) / (1.0 - DROP_P)  # [B, HID]

    w_event = np.asarray(inputs["w_event"], dtype=np.float32)
    wev = np.ascontiguousarray(
        w_event.reshape(8, 128, MD).transpose(1, 0, 2))
    w_fc1 = np.asarray(inputs["w_fc1"], dtype=np.float32)
    wfc1 = np.ascontiguousarray(
        w_fc1.reshape(2, 100, 3, 100).transpose(1, 0, 2, 3))
    w_ms = np.zeros((HID, 128), dtype=np.float32)
    w_ms[:, 0:LAT] = np.asarray(inputs["w_mu"], dtype=np.float32)
    w_ms[:, 64:64 + LAT] = np.asarray(inputs["w_sigma"], dtype=np.float32)
    wms = np.ascontiguousarray(w_ms.reshape(3, 100, 128).transpose(1, 0, 2))
    wmure = np.ascontiguousarray(np.asarray(inputs["w_mure"], dtype=np.float32))
    bmure = np.ascontiguousarray(
        np.asarray(inputs["b_mure"], dtype=np.float32)[None, :])
    bev = np.ascontiguousarray(
        np.asarray(inputs["b_event"], dtype=np.float32)[:, None])
    bfc1 = np.ascontiguousarray(
        np.asarray(inputs["b_fc1"], dtype=np.float32).reshape(3, 100).T)
    bms = np.zeros((128, 1), dtype=np.float32)
    bms[0:LAT, 0] = np.asarray(inputs["b_mu"], dtype=np.float32)
    bms[64:64 + LAT, 0] = np.asarray(inputs["b_sigma"], dtype=np.float32)
    gb = np.zeros((128, 2), dtype=np.float32)
    gb[0:LAT, 0] = np.asarray(inputs["gamma_mu"], dtype=np.float32)
    gb[64:64 + LAT, 0] = np.asarray(inputs["gamma_var"], dtype=np.float32)
    gb[0:LAT, 1] = np.asarray(inputs["beta_mu"], dtype=np.float32)
    gb[64:64 + LAT, 1] = np.asarray(inputs["beta_var"], dtype=np.float32)
    ident = np.eye(128, dtype=np.float32)

    shared = dict(wev=wev, wfc1=wfc1, wms=wms, wmure=wmure, bmure=bmure, bev=bev,
                  bfc1=bfc1, bms=bms, gb=gb, ident=ident)

    in_maps = []
    for c in range(NCORES):
        rs = slice(c * BC, (c + 1) * BC)
        e1t3 = e1[rs].T.reshape(8, 128, BC)   # [k, p, n]
        e2t3 = e2[rs].T.reshape(8, 128, BC)
        ee = np.ascontiguousarray(
            np.stack([e1t3, e2t3], axis=0).transpose(2, 1, 0, 3))
        maskt = np.ascontiguousarray(
            mask[rs].T.reshape(3, 100, BC).transpose(1, 0, 2))
        epst = np.ascontiguousarray(eps[rs].T)
        m = dict(shared)
        m.update(ee=ee, maskt=maskt, epst=epst)
        in_maps.append(m)
    return in_maps


def kernel(**inputs):
    global LAST_RESULTS
    from concourse.bass_utils import run_bass_kernel_spmd

    nc = _build()
    in_maps = _prep_inputs(inputs)
    res = run_bass_kernel_spmd(nc, in_maps, core_ids=list(range(NCORES)),
                               trace=TRACE)
    LAST_RESULTS = res
    outs = res.results
    logits = np.concatenate([outs[c]["logits_o"] for c in range(NCORES)], 0)
    mu = np.concatenate([outs[c]["mu_o"] for c in range(NCORES)], 0)
    lv = np.concatenate([outs[c]["lv_o"] for c in range(NCORES)], 0)
    return (logits, mu, lv, np.asarray(inputs["rel"]))


# revision 15
# speedup vs baseline: 1.1220x; 1.1220x over previous
"""Bayesian MuRP kernel for 8 TRN2 NeuronCores (Bass/Tile, SPMD data-parallel).

Strategy: data-parallel over batch (B=2048 -> 256 rows/core). The huge
lm_hidden_state is only ever read at 2 rows per batch element, so the gather
happens on host and only the gathered rows (~16 MB total) ship to HBM.
BatchNorm needs global batch stats -> tiny [128,2] AllReduce across the 8
cores inside the kernel. Everything else is per-core.

Layouts:
  - matmul chain runs feature-major (feature on partitions, batch on free)
  - the hyperbolic block runs batch-major with both 128-row halves fused
    side by side in the free dim: [128, h(2), r(4), 100]
  - premu/prelogvar packed on partitions 0-49 / 64-113 (base-partition
    rule: engine APs must start at multiples of 32)
"""

import numpy as np

B, S, H = 2048, 128, 1024
LAT, HID, MD, NR = 50, 300, 100, 4
BN_EPS = 1e-5
DROP_P = 0.5
NCORES = 8
BC = B // NCORES  # 256 rows per core
LV = 64           # partition offset of the logvar half

TRACE = False          # set True by test harness to capture NTFF profile
LAST_RESULTS = None    # BassKernelResults of the last run (for profiling)

_built = None


def _build():
    global _built
    if _built is not None:
        return _built

    import concourse.mybir as mybir
    import concourse.bacc as bacc
    import concourse.tile as tile

    f32 = mybir.dt.float32
    AF = mybir.ActivationFunctionType
    ALU = mybir.AluOpType
    X = mybir.AxisListType.X

    nc = bacc.Bacc("TRN2", target_bir_lowering=False, debug=False,
                   num_devices=NCORES)

    # ---- DRAM I/O ----
    ee_d = nc.dram_tensor("ee", [128, 8, 2, 256], f32, kind="ExternalInput")
    wev_d = nc.dram_tensor("wev", [128, 8, MD], f32, kind="ExternalInput")
    wfc1_d = nc.dram_tensor("wfc1", [100, 2, 3, 100], f32, kind="ExternalInput")
    wms_d = nc.dram_tensor("wms", [100, 3, 128], f32, kind="ExternalInput")
    wmure_d = nc.dram_tensor("wmure", [LAT, 800], f32, kind="ExternalInput")
    bmure_d = nc.dram_tensor("bmure", [1, 800], f32, kind="ExternalInput")
    maskt_d = nc.dram_tensor("maskt", [100, 3, 256], f32, kind="ExternalInput")
    epst_d = nc.dram_tensor("epst", [LAT, 256], f32, kind="ExternalInput")
    bev_d = nc.dram_tensor("bev", [MD, 1], f32, kind="ExternalInput")
    bfc1_d = nc.dram_tensor("bfc1", [100, 3], f32, kind="ExternalInput")
    bms_d = nc.dram_tensor("bms", [128, 1], f32, kind="ExternalInput")
    gb_d = nc.dram_tensor("gb", [128, 2], f32, kind="ExternalInput")
    ident_d = nc.dram_tensor("ident", [128, 128], f32, kind="ExternalInput")

    logits_d = nc.dram_tensor("logits_o", [BC, NR], f32, kind="ExternalOutput")
    mu_d = nc.dram_tensor("mu_o", [BC, LAT], f32, kind="ExternalOutput")
    lv_d = nc.dram_tensor("lv_o", [BC, LAT], f32, kind="ExternalOutput")

    with tile.TileContext(nc) as tc:
        with (
            tc.tile_pool(name="const", bufs=1) as cp,
            tc.tile_pool(name="act", bufs=1) as ap_,
            tc.tile_pool(name="ps", bufs=1, space="PSUM") as ps,
            tc.tile_pool(name="dram", bufs=1, space="DRAM") as dp,
        ):
            # ---------- constants / weights ----------
            wev = cp.tile([128, 8, MD], f32, tag="wev")
            nc.sync.dma_start(wev[:], wev_d[:])
            wfc1 = cp.tile([100, 2, 3, 100], f32, tag="wfc1")
            nc.sync.dma_start(wfc1[:], wfc1_d[:])
            wms = cp.tile([100, 3, 128], f32, tag="wms")
            nc.sync.dma_start(wms[:], wms_d[:])
            wmure = cp.tile([LAT, 800], f32, tag="wmure")
            nc.sync.dma_start(wmure[:], wmure_d[:])
            bmure = cp.tile([1, 800], f32, tag="bmure")
            nc.sync.dma_start(bmure[:], bmure_d[:])
            ones_t = cp.tile([1, 128], f32, tag="ones_t")
            nc.vector.memset(ones_t[:], 1.0)
            maskt = cp.tile([100, 3, 256], f32, tag="maskt")
            nc.sync.dma_start(maskt[:], maskt_d[:])
            epst = cp.tile([LAT, 256], f32, tag="epst")
            nc.sync.dma_start(epst[:], epst_d[:])
            bev = cp.tile([MD, 1], f32, tag="bev")
            nc.sync.dma_start(bev[:], bev_d[:])
            bfc1 = cp.tile([100, 3], f32, tag="bfc1")
            nc.sync.dma_start(bfc1[:], bfc1_d[:])
            bms = cp.tile([128, 1], f32, tag="bms")
            nc.sync.dma_start(bms[:], bms_d[:])
            gb = cp.tile([128, 2], f32, tag="gb")
            nc.sync.dma_start(gb[:], gb_d[:])
            ident = cp.tile([128, 128], f32, tag="ident")
            nc.sync.dma_start(ident[:], ident_d[:])

            # ---------- stage A: e1p/e2p = gathered rows @ w_event ----------
            ee = [cp.tile([128, 2, 256], f32, tag=f"ee{k}", name=f"ee{k}")
                  for k in range(8)]
            for k in range(8):
                nc.sync.dma_start(ee[k][:], ee_d[:, k, :, :])

            p_e1 = ps.tile([MD, 256], f32, tag="pA", bufs=2, name="p_e1")
            p_e2 = ps.tile([MD, 256], f32, tag="pA", bufs=2, name="p_e2")
            for k in range(8):
                nc.tensor.matmul(p_e1[:], wev[:, k, :], ee[k][:, 0, :],
                                 start=(k == 0), stop=(k == 7))
                nc.tensor.matmul(p_e2[:], wev[:, k, :], ee[k][:, 1, :],
                                 start=(k == 0), stop=(k == 7))
            e1pt = ap_.tile([MD, 256], f32, tag="e1pt")
            e2pt = ap_.tile([MD, 256], f32, tag="e2pt")
            nc.vector.tensor_scalar_add(e1pt[:], p_e1[:], bev[:])
            nc.vector.tensor_scalar_add(e2pt[:], p_e2[:], bev[:])

            # ---------- stage B: reps = relu(cat @ w_fc1 + b) * mask ----------
            repst = ap_.tile([100, 3, 256], f32, tag="repst")
            for j in range(3):
                p_f = ps.tile([100, 256], f32, tag="pB", bufs=2,
                              name=f"p_f{j}")
                nc.tensor.matmul(p_f[:], wfc1[:, 0, j, :], e1pt[:],
                                 start=True, stop=False)
                nc.tensor.matmul(p_f[:], wfc1[:, 1, j, :], e2pt[:],
                                 start=False, stop=True)
                # fused bias+relu: max(x+b, 0)
                nc.vector.tensor_scalar(repst[:, j, :], p_f[:],
                                        bfc1[:, j:j + 1], 0.0,
                                        ALU.add, ALU.max)
            nc.vector.tensor_mul(repst[:], repst[:], maskt[:])

            # ---------- stage C: pml = [premu; pad; prelogvar; pad] ----------
            p_ms = ps.tile([128, 256], f32, tag="pC", bufs=1, name="p_ms")
            for j in range(3):
                nc.tensor.matmul(p_ms[:], wms[:, j, :], repst[:, j, :],
                                 start=(j == 0), stop=(j == 2))
            pml = ap_.tile([128, 256], f32, tag="pml")
            nc.vector.tensor_scalar_add(pml[:], p_ms[:], bms[:])

            # ---------- stage D: partial batch stats + AllReduce ----------
            sums = ap_.tile([128, 2], f32, tag="sums")
            nc.vector.tensor_reduce(sums[:, 0:1], pml[:], X, ALU.add)
            sq_s = ap_.tile([128, 256], f32, tag="sqs")
            nc.vector.tensor_mul(sq_s[:], pml[:], pml[:])
            nc.vector.tensor_reduce(sums[:, 1:2], sq_s[:], X, ALU.add)
            cc_in = dp.tile([128, 2], f32, tag="ccin")
            cc_out = dp.tile([128, 2], f32, tag="ccout")
            nc.sync.dma_start(cc_in[:], sums[:])
            nc.gpsimd.collective_compute(
                "AllReduce", ALU.add,
                replica_groups=[list(range(NCORES))],
                ins=[cc_in.opt()],
                outs=[cc_out.opt()],
            )
            gsums = ap_.tile([128, 2], f32, tag="gsums")
            nc.sync.dma_start(gsums[:], cc_out[:])

            # ---------- overlap window: transposes + v-path (CC-independent)
            e1p4 = ap_.tile([128, 2, 4, 100], f32, tag="e1p4")
            e2p4 = ap_.tile([128, 2, 4, 100], f32, tag="e2p4")
            for src, dst4 in ((e1pt, e1p4), (e2pt, e2p4)):
                for h in range(2):
                    ptr = ps.tile([128, 100], f32, tag="pT", bufs=1,
                                  name=f"ptr_{h}")
                    nc.tensor.transpose(ptr[:], src[:, h * 128:(h + 1) * 128],
                                        ident[0:100, 0:100])
                    bc = ptr[:].unsqueeze(1).broadcast_to([128, 4, 100])
                    nc.vector.tensor_copy(dst4[:, h], bc)

            # v = expmap0(e2p): per-half scalars [128, 2]
            vsc = ap_.tile([128, 8], f32, tag="vsc")
            n2v = vsc[:, 0:2]
            thv = vsc[:, 2:4]
            facv = vsc[:, 4:6]
            x2v = vsc[:, 6:8]
            sq1 = ap_.tile([128, 2, 100], f32, tag="sq1")
            nc.gpsimd.tensor_mul(sq1[:], e2p4[:, :, 0, :], e2p4[:, :, 0, :])
            nc.vector.tensor_reduce(n2v, sq1[:], X, ALU.add)
            nc.scalar.activation(n2v, n2v, AF.Sqrt)
            nc.vector.tensor_scalar_max(n2v, n2v, 1e-15)
            nc.scalar.activation(thv, n2v, AF.Tanh)
            nc.vector.reciprocal(facv, n2v)
            nc.vector.tensor_mul(facv, thv, facv)        # tanh(n)/n
            nc.vector.tensor_mul(x2v, thv, thv)

            # ---------- stage E: global stats -> bn affine ----------
            st = ap_.tile([128, 6], f32, tag="st")  # m, ex2, var, sd, a, c
            nc.vector.tensor_scalar_mul(st[:, 0:2], gsums[:], 1.0 / B)
            nc.vector.tensor_mul(st[:, 2:3], st[:, 0:1], st[:, 0:1])
            nc.vector.tensor_sub(st[:, 2:3], st[:, 1:2], st[:, 2:3])
            nc.vector.tensor_scalar_add(st[:, 2:3], st[:, 2:3], BN_EPS)
            nc.scalar.activation(st[:, 3:4], st[:, 2:3], AF.Sqrt)
            nc.vector.reciprocal(st[:, 3:4], st[:, 3:4])
            nc.vector.tensor_mul(st[:, 4:5], gb[:, 0:1], st[:, 3:4])
            nc.vector.tensor_mul(st[:, 5:6], st[:, 0:1], st[:, 4:5])
            nc.vector.tensor_sub(st[:, 5:6], gb[:, 1:2], st[:, 5:6])
            bn = ap_.tile([128, 256], f32, tag="bn")
            nc.vector.tensor_scalar(bn[:], pml[:], st[:, 4:5], st[:, 5:6],
                                    ALU.mult, ALU.add)

            # mu/logvar outputs via PE transpose
            for h in range(2):
                ptm = ps.tile([128, 128], f32, tag="pT", bufs=1,
                              name=f"ptm_{h}")
                nc.tensor.transpose(ptm[:], bn[:, h * 128:(h + 1) * 128],
                                    ident[:])
                mlv = ap_.tile([128, 128], f32, tag="mlv", name=f"mlv{h}")
                nc.vector.tensor_copy(mlv[:], ptm[:])
                nc.sync.dma_start(mu_d[h * 128:(h + 1) * 128, :],
                                  mlv[:, 0:LAT])
                nc.sync.dma_start(lv_d[h * 128:(h + 1) * 128, :],
                                  mlv[:, LV:LV + LAT])

            # ---------- stage F: zt ----------
            zt = ap_.tile([LAT, 256], f32, tag="zt")
            sig = ap_.tile([LAT, 256], f32, tag="sig")
            nc.scalar.activation(sig[:], bn[LV:LV + LAT, :], AF.Exp, scale=0.5)
            nc.vector.tensor_mul(zt[:], sig[:], epst[:])
            nc.vector.tensor_add(zt[:], zt[:], bn[0:LAT, :])

            # ---------- stage G: tp = tanh(z @ w_mure + b) batch-major ----
            tpWu = ap_.tile([128, 2, 4, 100], f32, tag="tpWu")
            tprvh = ap_.tile([128, 2, 4, 100], f32, tag="tprvh")
            for h in range(2):
                hs = slice(h * 128, (h + 1) * 128)
                for seg, dst in ((0, tpWu), (1, tprvh)):
                    pg = ps.tile([128, 400], f32, tag="pG", bufs=2,
                                 name=f"pg{h}{seg}")
                    cs = slice(seg * 400, (seg + 1) * 400)
                    nc.tensor.matmul(pg[:], zt[:, hs], wmure[:, cs],
                                     start=True, stop=False)
                    nc.tensor.matmul(pg[:], ones_t[:], bmure[:, cs],
                                     start=False, stop=True)
                    nc.scalar.activation(dst[:, h], pg[:], AF.Tanh)

            # ---------- stage H: hyperbolic block, halves fused ----------
            def bc8(t8):  # [128,(2,4)] -> [128,2,4,100] broadcast view
                return t8.unsqueeze(3).broadcast_to([128, 2, 4, 100])

            def bcv(t2):  # [128,2] -> [128,2,4,100] broadcast view
                return (t2.unsqueeze(2).unsqueeze(3)
                        .broadcast_to([128, 2, 4, 100]))

            s4 = ap_.tile([128, 2, 4, 16], f32, tag="s4")
            n2u = s4[:, :, :, 0]
            thu = s4[:, :, :, 1]
            facu = s4[:, :, :, 2]
            x2u = s4[:, :, :, 3]
            y2 = s4[:, :, :, 4]
            sxy = s4[:, :, :, 5]
            t2s = s4[:, :, :, 6]
            cA = s4[:, :, :, 7]
            den = s4[:, :, :, 8]
            y2p = s4[:, :, :, 9]
            sp = s4[:, :, :, 10]
            t2sp = s4[:, :, :, 11]
            cAp = s4[:, :, :, 12]
            denp = s4[:, :, :, 13]
            nn2 = s4[:, :, :, 14]
            cBp = s4[:, :, :, 15]

            # u_in = e1p * Wu ; norms per (h, r)
            u_in = ap_.tile([128, 2, 4, 100], f32, tag="uin")
            nc.vector.tensor_mul(u_in[:], e1p4[:], tpWu[:])
            squ = ap_.tile([128, 2, 4, 100], f32, tag="squ")
            nc.vector.tensor_mul(squ[:], u_in[:], u_in[:])
            nc.vector.tensor_reduce(n2u, squ[:], X, ALU.add)
            nc.scalar.activation(n2u, n2u, AF.Sqrt)
            nc.vector.tensor_scalar_max(n2u, n2u, 1e-15)
            nc.scalar.activation(thu, n2u, AF.Tanh)
            nc.vector.reciprocal(facu, n2u)
            nc.vector.tensor_mul(facu, thu, facu)
            nc.vector.tensor_mul(x2u, thu, thu)
            nc.vector.tensor_scalar(cBp, x2u, -1.0, 1.0, ALU.mult, ALU.add)

            # v_m = mobius_add(v, rvh); v = facv*e2p (not materialized)
            sqr = ap_.tile([128, 2, 4, 100], f32, tag="sqr")
            nc.gpsimd.tensor_mul(sqr[:], tprvh[:], tprvh[:])
            nc.vector.tensor_reduce(y2, sqr[:], X, ALU.add)
            pr = ap_.tile([128, 2, 4, 100], f32, tag="pr")
            nc.gpsimd.tensor_mul(pr[:], e2p4[:], tprvh[:])
            nc.vector.tensor_reduce(sxy, pr[:], X, ALU.add)
            # sxy currently = sum(e2p*rvh); scale per-half by facv
            fv2 = vsc[:, 4:6].unsqueeze(2).broadcast_to([128, 2, 4])
            nc.vector.tensor_mul(sxy, sxy, fv2)          # xy
            nc.vector.tensor_scalar(t2s, sxy, 2.0, 1.0, ALU.mult, ALU.add)
            nc.vector.tensor_add(cA, t2s, y2)
            xv2 = vsc[:, 6:8].unsqueeze(2).broadcast_to([128, 2, 4])
            nc.vector.tensor_mul(den, y2, xv2)
            nc.vector.tensor_add(den, den, t2s)
            nc.vector.tensor_scalar_max(den, den, 1e-15)
            nc.vector.reciprocal(den, den)

            nc.vector.tensor_mul(cA, cA, fv2)            # coefA*facv
            num = ap_.tile([128, 2, 4, 100], f32, tag="num")
            nc.vector.tensor_mul(num[:], e2p4[:], bc8(cA))
            vm = ap_.tile([128, 2, 4, 100], f32, tag="vm")
            nc.vector.tensor_mul(vm[:], tprvh[:], bcv(vsc[:, 6:8]))
            # cBv = 1 - x2v: use scalar_tensor form: vm = rvh*(1-x2v) =
            # rvh - rvh*x2v  -> do sub from tprvh
            nc.vector.tensor_sub(vm[:], tprvh[:], vm[:])
            nc.vector.tensor_add(num[:], num[:], vm[:])
            nc.vector.tensor_mul(vm[:], num[:], bc8(den))  # v_m

            # pdist(u_m, v_m); u_m = facu (x) u_in (not materialized)
            nc.gpsimd.tensor_mul(sqr[:], vm[:], vm[:])
            nc.vector.tensor_reduce(y2p, sqr[:], X, ALU.add)
            nc.gpsimd.tensor_mul(pr[:], u_in[:], vm[:])
            nc.vector.tensor_reduce(sp, pr[:], X, ALU.add)
            nc.vector.tensor_mul(sp, sp, facu)           # u_m . v_m
            nc.vector.tensor_scalar(t2sp, sp, -2.0, 1.0, ALU.mult, ALU.add)
            nc.vector.tensor_add(cAp, t2sp, y2p)
            nc.vector.tensor_mul(denp, x2u, y2p)
            nc.vector.tensor_add(denp, denp, t2sp)
            nc.vector.tensor_scalar_max(denp, denp, 1e-15)
            nc.vector.reciprocal(denp, denp)

            nc.vector.tensor_mul(cAp, cAp, facu)         # coefAp*facu
            nc.vector.tensor_mul(num[:], u_in[:], bc8(cAp))
            nc.vector.tensor_mul(vm[:], vm[:], bc8(cBp))
            nc.vector.tensor_sub(num[:], vm[:], num[:])  # num_p
            nc.vector.tensor_mul(num[:], num[:], num[:])
            nc.vector.tensor_reduce(nn2, num[:], X, ALU.add)

            lg = ap_.tile([128, 2, 4, 4], f32, tag="lg")
            n_ = lg[:, :, :, 0]
            p_ = lg[:, :, :, 1]
            q_ = lg[:, :, :, 2]
            o_ = lg[:, :, :, 3]
            nc.scalar.activation(n_, nn2, AF.Sqrt)
            nc.vector.tensor_mul(n_, n_, denp)           # n = |num|/den
            nc.vector.tensor_scalar_min(n_, n_, 1.0 - 1e-5)
            nc.vector.tensor_scalar(p_, n_, 1.0, 1.0, ALU.mult, ALU.add)
            nc.vector.tensor_scalar(q_, n_, -1.0, 1.0, ALU.mult, ALU.add)
            nc.vector.reciprocal(q_, q_)
            nc.vector.tensor_mul(p_, p_, q_)
            nc.scalar.activation(o_, p_, AF.Ln)
            for h in range(2):
                nc.sync.dma_start(logits_d[h * 128:(h + 1) * 128, :],
                                  lg[:, h, :, 3])

    nc.compile()
    _built = nc
    return nc


def _prep_inputs(inputs):
    lm = np.asarray(inputs["lm_hidden_state"], dtype=np.float32)
    ep1 = np.asarray(inputs["epos_1"]).astype(np.int64)
    ep2 = np.asarray(inputs["epos_2"]).astype(np.int64)
    eps = np.asarray(inputs["eps"], dtype=np.float32)
    drop_u = np.asarray(inputs["drop_u"], dtype=np.float32)

    bidx = np.arange(B)
    e1 = lm[bidx, ep1]          # [B, H]
    e2 = lm[bidx, ep2]
    mask = (drop_u > DROP_P).astype(np.float32) / (1.0 - DROP_P)  # [B, HID]

    w_event = np.asarray(inputs["w_event"], dtype=np.float32)
    wev = np.ascontiguousarray(
        w_event.reshape(8, 128, MD).transpose(1, 0, 2))
    w_fc1 = np.asarray(inputs["w_fc1"], dtype=np.float32)
    wfc1 = np.ascontiguousarray(
        w_fc1.reshape(2, 100, 3, 100).transpose(1, 0, 2, 3))
    w_ms = np.zeros((HID, 128), dtype=np.float32)
    w_ms[:, 0:LAT] = np.asarray(inputs["w_mu"], dtype=np.float32)
    w_ms[:, LV:LV + LAT] = np.asarray(inputs["w_sigma"], dtype=np.float32)
    wms = np.ascontiguousarray(w_ms.reshape(3, 100, 128).transpose(1, 0, 2))
    wmure = np.ascontiguousarray(np.asarray(inputs["w_mure"], dtype=np.float32))
    bmure = np.ascontiguousarray(
        np.asarray(inputs["b_mure"], dtype=np.float32)[None, :])
    bev = np.ascontiguousarray(
        np.asarray(inputs["b_event"], dtype=np.float32)[:, None])
    bfc1 = np.ascontiguousarray(
        np.asarray(inputs["b_fc1"], dtype=np.float32).reshape(3, 100).T)
    bms = np.zeros((128, 1), dtype=np.float32)
    bms[0:LAT, 0] = np.asarray(inputs["b_mu"], dtype=np.float32)
    bms[LV:LV + LAT, 0] = np.asarray(inputs["b_sigma"], dtype=np.float32)
    gb = np.zeros((128, 2), dtype=np.float32)
    gb[0:LAT, 0] = np.asarray(inputs["gamma_mu"], dtype=np.float32)
    gb[LV:LV + LAT, 0] = np.asarray(inputs["gamma_var"], dtype=np.float32)
    gb[0:LAT, 1] = np.asarray(inputs["beta_mu"], dtype=np.float32)
    gb[LV:LV + LAT, 1] = np.asarray(inputs["beta_var"], dtype=np.float32)
    ident = np.eye(128, dtype=np.float32)

    shared = dict(wev=wev, wfc1=wfc1, wms=wms, wmure=wmure, bmure=bmure, bev=bev,
                  bfc1=bfc1, bms=bms, gb=gb, ident=ident)

    in_maps = []
    for c in range(NCORES):
        rs = slice(c * BC, (c + 1) * BC)
        e1t3 = e1[rs].T.reshape(8, 128, BC)   # [k, p, n]
        e2t3 = e2[rs].T.reshape(8, 128, BC)
        ee = np.ascontiguousarray(
            np.stack([e1t3, e2t3], axis=0).transpose(2, 1, 0, 3))
        maskt = np.ascontiguousarray(
            mask[rs].T.reshape(3, 100, BC).transpose(1, 0, 2))
        epst = np.ascontiguousarray(eps[rs].T)
        m = dict(shared)
        m.update(ee=ee, maskt=maskt, epst=epst)
        in_maps.append(m)
    return in_maps


def kernel(**inputs):
    global LAST_RESULTS
    from concourse.bass_utils import run_bass_kernel_spmd

    nc = _build()
    in_maps = _prep_inputs(inputs)
    res = run_bass_kernel_spmd(nc, in_maps, core_ids=list(range(NCORES)),
                               trace=TRACE)
    LAST_RESULTS = res
    outs = res.results
    logits = np.concatenate([outs[c]["logits_o"] for c in range(NCORES)], 0)
    mu = np.concatenate([outs[c]["mu_o"] for c in range(NCORES)], 0)
    lv = np.concatenate([outs[c]["lv_o"] for c in range(NCORES)], 0)
    return (logits, mu, lv, np.asarray(inputs["rel"]))
